# revision 9
# baseline (speedup 1.0000x reference)
"""AdaptiveConv2DMod kernel for 8 TRN2 NeuronCores.

Data-parallel over batch: B=16 -> 2 samples per core.

All transforms run host-side in fp32 numpy (mod/kernel_mod/weights are
host-visible); the device is a pure Winograd-domain batched GEMM:

- Weight math (softmax kernel mix, (1+mod) modulation, demod rsqrt) and
  the F(2,3) 1D Winograd weight transform U = G g (along kx) happen on
  host; each core gets its two samples' U pre-transposed to the matmul
  lhsT layout [b, co, i(128p), ci, s(4), ky(3), o(128)] bf16.
- The fmap is padded (rows+cols) and column-transformed on host into
  V[s] = B^T d (4 Winograd points per 2 output columns), shipped as
  [b, ci, s, ch(128p), 66 rows, 32 tx] bf16.
- Device: 384 matmuls M[s] += U[s,ky].T @ V[s] (shifted rows give the
  direct-ky accumulation; bf16 in / fp32 PSUM), 1.5x less PE work than
  direct 3x3 conv. M drains PSUM -> bf16 SBUF on DVE -> DMA out.
- Host: output transform out = A^T M (3-term combines) in fp32.

PE floor: 384 x 128x128x512 matmuls ~= 82us. HAM warmup dummies keep
the PE clock gate at 8/8 before real work. Weights ride the sync HWDGE
ring, V rides gpsimd SWDGE, M output rides sync behind the weights.
Loop s -> ci -> ky -> nt(4) gives 4-matmul ldweights runs (deduped)
and per-s PSUM drains that overlap the next s-block.
"""

from contextlib import ExitStack

import ml_dtypes
import numpy as np

import concourse.bass as bass
import concourse.mybir as mybir
import concourse.tile as tile
from concourse import bacc
from concourse.bass_utils import run_bass_kernel_spmd

F32 = mybir.dt.float32
BF16 = mybir.dt.bfloat16
BF16_NP = ml_dtypes.bfloat16

N_CORES = 8
B_LOC = 2          # samples per core
C = 256            # input channels (I)
O = 256            # output channels
H = W = 64
NK = 4             # num base kernels
CI = 2             # input channel chunks of 128
CO = 2             # output channel chunks of 128
NS = 4             # winograd points per 2 output cols
KY = 3             # direct taps along y
TX = W // 2        # winograd tiles per row
VR = H + 2         # padded rows in V
NT = 4             # row tiles (16 rows x 32 tx = 512 free)
RPT = H // NT      # rows per tile
WCOLS = CI * NS * KY * 128   # wt free size (3072)
VCOLS = VR * TX              # v free size (2112)


def _build_nc(repeat=1):
    nc = bacc.Bacc("TRN2", target_bir_lowering=False, debug=False,
                   num_devices=N_CORES)
    wt = nc.declare_dram_parameter("wt", [B_LOC, CO, 128, WCOLS],
                                   BF16, isOutput=False)
    v = nc.declare_dram_parameter("v", [B_LOC, CI, NS, 128, VCOLS],
                                  BF16, isOutput=False)
    out = nc.declare_dram_parameter("out", [B_LOC, CO, NS, 128, H * TX],
                                    BF16, isOutput=True)

    with ExitStack() as ctx:
        tc = ctx.enter_context(tile.TileContext(nc))
        pools = _make_pools(ctx, tc)
        for _ in range(repeat):
            _build_body(tc, pools, wt.ap(), v.ap(), out.ap())
    _dedupe_ldweights(nc)
    nc.compile()
    return nc


def _dedupe_ldweights(nc):
    """Remove PE weight reloads that are byte-identical to the previous
    Ldweights and carry no semaphore waits/updates (the split emits one
    Ldweights per matmul even when the stationary operand is unchanged)."""
    removed = 0
    pe = mybir.EngineType.PE
    for blk in nc.main_func.blocks:
        last_key = None
        keep = []
        for inst in blk.instructions:
            tn = type(inst).__name__
            eng = getattr(inst, "engine", None)
            if tn == "InstLdweights":
                key = repr(inst.ins)
                if (key == last_key and inst.sync_info is None):
                    removed += 1
                    continue
                last_key = key
            elif tn == "InstMatmult":
                pass
            elif eng == pe:
                last_key = None
            keep.append(inst)
        blk.instructions[:] = keep
    return removed


def _make_pools(ctx, tc):
    return {
        "wt": ctx.enter_context(tc.tile_pool(name="wt", bufs=B_LOC * CO)),
        "v": ctx.enter_context(tc.tile_pool(name="v", bufs=B_LOC * CI * NS)),
        "outp": ctx.enter_context(tc.tile_pool(name="outp", bufs=8)),
        "psconv": ctx.enter_context(
            tc.tile_pool(name="psconv", bufs=8, space="PSUM")),
    }


def _build_body(tc, pools, wt_dram, v_dram, out_dram):
    nc = tc.nc
    wtp = pools["wt"]
    vp = pools["v"]
    outp = pools["outp"]
    psconv = pools["psconv"]

    w_T = [[None] * CO for _ in range(B_LOC)]
    v_t = [[[None] * NS for _ in range(CI)] for _ in range(B_LOC)]

    def load_wt(b, co, ci=None, eng=None):
        if ci is None or ci == 0:
            t = wtp.tile([128, WCOLS], BF16, tag="wt", name=f"wT{b}_{co}")
            w_T[b][co] = t
        t = w_T[b][co]
        eng = eng or nc.gpsimd
        if ci is None:
            eng.dma_start(out=t[:], in_=wt_dram[b, co])
        else:
            cw = NS * KY * 128
            eng.dma_start(out=t[:, ci * cw:(ci + 1) * cw],
                          in_=wt_dram[b, co, :, ci * cw:(ci + 1) * cw])

    def load_v(b, ci, s):
        t = vp.tile([128, VCOLS], BF16, tag="v", name=f"v{b}_{ci}_{s}")
        nc.gpsimd.dma_start(out=t[:], in_=v_dram[b, ci, s])
        v_t[b][ci][s] = t

    # HAM warmup: dummy matmuls keep PE busy from kernel start so the
    # clock gate is at 8/8 when the first real matmul issues (needs
    # >=3.4us of sustained PE busy; 9 cold matmuls ~= 3.8us). The dummy
    # PSUM slot is released before conv(0,0) claims its 8th bank.
    wz = wtp.tile([128, 512], BF16, tag="wz", bufs=1)
    nc.gpsimd.memset(wz[:], 0.0)
    psd = psconv.tile([128, 512], F32, tag="ps", name="psdummy")
    for _ in range(9):
        nc.tensor.matmul(psd[:], wz[:, 0:128], wz[:], start=True, stop=True)

    # input DMAs in consumption order. Only the startup-critical wt(0,0)
    # rides the sync HWDGE ring (outputs join it later); everything else
    # streams on gpsimd SWDGE so the early weight traffic doesn't starve
    # v(0,*) (HBM + SDMA engines are shared across rings).
    load_wt(0, 0, ci=0, eng=nc.sync)
    load_v(0, 0, 0)
    load_wt(0, 0, ci=1, eng=nc.sync)
    load_v(0, 1, 0)
    for s in range(1, NS):
        load_v(0, 0, s)
        load_v(0, 1, s)
    load_wt(0, 1)
    load_v(1, 0, 0)
    load_v(1, 1, 0)
    load_wt(1, 0)
    for s in range(1, NS):
        load_v(1, 0, s)
        load_v(1, 1, s)
    load_wt(1, 1)

    # ---- winograd-domain GEMM: M[s] = sum_{ci,ky} U[ci,s,ky].T @ V[s] ------
    def drain(b, co, s, nt, ps, last=False):
        # steady state: DVE cast + sync HWDGE out. For the final block the
        # cast/DMA chains split across vector/scalar engines and the two
        # HWDGE rings so the tail is two half-length chains, not one.
        ot = outp.tile([128, RPT * TX], BF16, tag="ot")
        if last and nt % 2 == 1:
            nc.scalar.copy(ot[:], ps[:])
            dma_eng = nc.scalar
        else:
            nc.vector.tensor_copy(ot[:], ps[:])
            dma_eng = nc.sync
        dma_eng.dma_start(
            out=out_dram[b, co, s, :, nt * RPT * TX:(nt + 1) * RPT * TX],
            in_=ot[:])

    def conv(b, co, last=False):
        for s in range(NS):
            ps = [psconv.tile([128, RPT * TX], F32, tag="ps",
                              name=f"ps{b}_{co}_{s}_{nt}")
                  for nt in range(NT)]
            for ci in range(CI):
                for ky in range(KY):
                    lhsT = w_T[b][co][:, ((ci * NS + s) * KY + ky) * 128:
                                      ((ci * NS + s) * KY + ky + 1) * 128]
                    for nt in range(NT):
                        r0 = nt * RPT + ky
                        rhs = v_t[b][ci][s][:, r0 * TX:(r0 + RPT) * TX]
                        nc.tensor.matmul(
                            ps[nt][:], lhsT, rhs,
                            start=(ci == 0 and ky == 0),
                            stop=(ci == CI - 1 and ky == KY - 1))
            for nt in range(NT):
                drain(b, co, s, nt, ps[nt], last=(last and s == NS - 1))

    for b in range(B_LOC):
        for co in range(CO):
            conv(b, co, last=(b == B_LOC - 1 and co == CO - 1))


_NC_CACHE = {}


def _get_nc(repeat=1):
    key = repeat
    if key not in _NC_CACHE:
        _NC_CACHE[key] = _build_nc(repeat)
    return _NC_CACHE[key]


def _prep_host(fmap, mod, kernel_mod, weights):
    """Host-side fp32 weight math + winograd transforms (F(2,3) along x)."""
    B = fmap.shape[0]
    # softmax over the NK base kernels
    e = np.exp(kernel_mod - kernel_mod.max(axis=-1, keepdims=True))
    attn = (e / e.sum(axis=-1, keepdims=True)).astype(np.float32)   # [B, NK]
    w = np.einsum('bn,noikl->boikl', attn, weights)     # [B, O, C, 3, 3]
    w = w * (mod[:, None, :, None, None] + 1.0)
    denom = np.clip((w * w).sum(axis=(2, 3, 4), keepdims=True), 1e-8, None)
    w = w / np.sqrt(denom)
    # weight transform U = G g along kx: [B, O, C, ky, s]
    U = np.stack([w[..., 0],
                  0.5 * (w[..., 0] + w[..., 1] + w[..., 2]),
                  0.5 * (w[..., 0] - w[..., 1] + w[..., 2]),
                  w[..., 2]], axis=-1)
    # lhsT layout: [b, co, i(128p), ci, s, ky, o(128)]
    wt = U.reshape(B, CO, 128, CI, 128, KY, NS)
    wt = wt.transpose(0, 1, 4, 3, 6, 5, 2)       # [b, co, i, ci, s, ky, o]
    wt = np.ascontiguousarray(wt).reshape(B, CO, 128, WCOLS).astype(BF16_NP)
    # input transform V[s] = B^T d along padded cols, rows padded for ky
    dp = np.zeros((B, C, VR, W + 2), dtype=np.float32)
    dp[:, :, 1:H + 1, 1:W + 1] = fmap
    V = np.stack([dp[..., 0:2 * TX:2] - dp[..., 2:2 * TX + 2:2],
                  dp[..., 1:2 * TX + 1:2] + dp[..., 2:2 * TX + 2:2],
                  dp[..., 2:2 * TX + 2:2] - dp[..., 1:2 * TX + 1:2],
                  dp[..., 1:2 * TX + 1:2] - dp[..., 3:2 * TX + 3:2]],
                 axis=2)                          # [B, C, s, VR, TX]
    V = V.reshape(B, CI, 128, NS, VR * TX).transpose(0, 1, 3, 2, 4)
    V = np.ascontiguousarray(V).astype(BF16_NP)   # [B, CI, s, 128, VCOLS]
    return wt, V


def _make_in_maps(wt, V):
    in_maps = []
    for c in range(N_CORES):
        s = slice(c * B_LOC, (c + 1) * B_LOC)
        in_maps.append({
            "wt": np.ascontiguousarray(wt[s]),
            "v": np.ascontiguousarray(V[s]),
        })
    return in_maps


def kernel(fmap, mod, kernel_mod, weights, _trace=False):
    fmap = np.asarray(fmap, dtype=np.float32)
    mod = np.asarray(mod, dtype=np.float32)
    kernel_mod = np.asarray(kernel_mod, dtype=np.float32)
    weights = np.asarray(weights, dtype=np.float32)

    wt, V = _prep_host(fmap, mod, kernel_mod, weights)
    nc = _get_nc()
    in_maps = _make_in_maps(wt, V)
    res = run_bass_kernel_spmd(nc, in_maps, list(range(N_CORES)), trace=_trace)
    B = fmap.shape[0]
    M = np.concatenate([res.results[c]["out"] for c in range(N_CORES)],
                       axis=0).astype(np.float32)
    M = M.reshape(B, CO, NS, 128, H, TX)          # [b, co, s, o, y, tx]
    out = np.empty((B, CO, 128, H, W), dtype=np.float32)
    out[..., 0::2] = M[:, :, 0] + M[:, :, 1] + M[:, :, 2]
    out[..., 1::2] = M[:, :, 1] - M[:, :, 2] - M[:, :, 3]
    out = out.reshape(B, O, H, W)
    if _trace:
        kernel.last_results = res
    return out


# revision 10
# speedup vs baseline: 1.1264x; 1.1264x over previous
"""AdaptiveConv2DMod kernel for 8 TRN2 NeuronCores.

Data-parallel over batch: B=16 -> 2 samples per core.

All transforms run host-side in fp32 numpy (mod/kernel_mod/weights are
host-visible); the device is a pure Winograd-domain batched GEMM:

- Weight math (softmax kernel mix, (1+mod) modulation, demod rsqrt) and
  the F(2,3) 1D Winograd weight transform U = G g (along kx) happen on
  host; each core gets its two samples' U pre-transposed to the matmul
  lhsT layout [b, co, i(128p), ci, s(4), ky(3), o(128)] bf16.
- The fmap is padded (rows+cols) and column-transformed on host into
  V[s] = B^T d (4 Winograd points per 2 output columns), shipped as
  [b, ci, s, ch(128p), 66 rows, 32 tx] bf16.
- Device: 384 matmuls M[s] += U[s,ky].T @ V[s] (shifted rows give the
  direct-ky accumulation; bf16 in / fp32 PSUM), 1.5x less PE work than
  direct 3x3 conv. M drains PSUM -> bf16 SBUF on DVE -> DMA out.
- Host: output transform out = A^T M (3-term combines) in fp32.

PE floor: 384 x 128x128x512 matmuls ~= 82us. HAM warmup dummies keep
the PE clock gate at 8/8 before real work. Weights ride the sync HWDGE
ring, V rides gpsimd SWDGE, M output rides sync behind the weights.
Loop s -> ci -> ky -> nt(4) gives 4-matmul ldweights runs (deduped)
and per-s PSUM drains that overlap the next s-block.
"""

from contextlib import ExitStack

import ml_dtypes
import numpy as np

import concourse.bass as bass
import concourse.mybir as mybir
import concourse.tile as tile
from concourse import bacc
from concourse.bass_utils import run_bass_kernel_spmd

F32 = mybir.dt.float32
BF16 = mybir.dt.bfloat16
BF16_NP = ml_dtypes.bfloat16

N_CORES = 8
B_LOC = 2          # samples per core
C = 256            # input channels (I)
O = 256            # output channels
H = W = 64
NK = 4             # num base kernels
CI = 2             # input channel chunks of 128
CO = 2             # output channel chunks of 128
NS = 4             # winograd points per 2 output cols
KY = 3             # direct taps along y
TX = W // 2        # winograd tiles per row
VR = H + 2         # padded rows in V
NT = 4             # row tiles (16 rows x 32 tx = 512 free)
RPT = H // NT      # rows per tile
WCOLS = CI * NS * KY * 128   # wt free size (3072)
VCOLS = VR * TX              # v free size (2112)


def _build_nc(repeat=1):
    nc = bacc.Bacc("TRN2", target_bir_lowering=False, debug=False,
                   num_devices=N_CORES)
    wt = nc.declare_dram_parameter("wt", [B_LOC, CO, 128, WCOLS],
                                   BF16, isOutput=False)
    v = nc.declare_dram_parameter("v", [B_LOC, CI, NS, 128, VCOLS],
                                  BF16, isOutput=False)
    out = nc.declare_dram_parameter("out", [B_LOC, CO, NS, 128, H * TX],
                                    BF16, isOutput=True)

    with ExitStack() as ctx:
        tc = ctx.enter_context(tile.TileContext(nc))
        pools = _make_pools(ctx, tc)
        for _ in range(repeat):
            _build_body(tc, pools, wt.ap(), v.ap(), out.ap())
    _dedupe_ldweights(nc)
    nc.compile()
    return nc


def _dedupe_ldweights(nc):
    """Remove PE weight reloads that are byte-identical to the previous
    Ldweights and carry no semaphore waits/updates (the split emits one
    Ldweights per matmul even when the stationary operand is unchanged)."""
    removed = 0
    pe = mybir.EngineType.PE
    for blk in nc.main_func.blocks:
        last_key = None
        keep = []
        for inst in blk.instructions:
            tn = type(inst).__name__
            eng = getattr(inst, "engine", None)
            if tn == "InstLdweights":
                key = repr(inst.ins)
                if (key == last_key and inst.sync_info is None):
                    removed += 1
                    continue
                last_key = key
            elif tn == "InstMatmult":
                pass
            elif eng == pe:
                last_key = None
            keep.append(inst)
        blk.instructions[:] = keep
    return removed


def _make_pools(ctx, tc):
    return {
        "wt": ctx.enter_context(tc.tile_pool(name="wt", bufs=B_LOC * CO)),
        "v": ctx.enter_context(tc.tile_pool(name="v", bufs=B_LOC * CI * NS)),
        "outp": ctx.enter_context(tc.tile_pool(name="outp", bufs=8)),
        "psconv": ctx.enter_context(
            tc.tile_pool(name="psconv", bufs=8, space="PSUM")),
    }


def _build_body(tc, pools, wt_dram, v_dram, out_dram):
    nc = tc.nc
    wtp = pools["wt"]
    vp = pools["v"]
    outp = pools["outp"]
    psconv = pools["psconv"]

    w_T = [[None] * CO for _ in range(B_LOC)]
    v_t = [[[None] * NS for _ in range(CI)] for _ in range(B_LOC)]

    CW = NS * KY * 128        # wt cols per ci block
    SW = KY * 128             # wt cols per (ci, s) block

    def wt_tile(b, co):
        t = wtp.tile([128, WCOLS], BF16, tag="wt", name=f"wT{b}_{co}")
        w_T[b][co] = t
        return t

    def load_wt_block(b, co, ci, s, eng):
        c0 = ci * CW + s * SW
        eng.dma_start(out=w_T[b][co][:, c0:c0 + SW],
                      in_=wt_dram[b, co, :, c0:c0 + SW])

    def load_wt(b, co, eng=None):
        t = wt_tile(b, co)
        (eng or nc.gpsimd).dma_start(out=t[:], in_=wt_dram[b, co])

    def v_tile(b, ci, s):
        t = vp.tile([128, VCOLS], BF16, tag="v", name=f"v{b}_{ci}_{s}")
        v_t[b][ci][s] = t
        return t

    def load_v(b, ci, s, rows=None, eng=None):
        t = v_t[b][ci][s] if v_t[b][ci][s] is not None else v_tile(b, ci, s)
        r0, r1 = rows if rows is not None else (0, VR)
        (eng or nc.gpsimd).dma_start(
            out=t[:, r0 * TX:r1 * TX],
            in_=v_dram[b, ci, s, :, r0 * TX:r1 * TX])

    # HAM warmup: dummy matmuls keep PE busy from kernel start so the
    # clock gate is at 8/8 when the first real matmul issues (needs
    # ~3.4us of sustained PE busy; 8 cold matmuls ~= 3.4us). The dummy
    # PSUM slot is released before conv(0,0) claims its 8th bank.
    wz = wtp.tile([128, 512], BF16, tag="wz", bufs=1)
    nc.gpsimd.memset(wz[:], 0.0)
    psd = psconv.tile([128, 512], F32, tag="ps", name="psdummy")
    for _ in range(8):
        nc.tensor.matmul(psd[:], wz[:, 0:128], wz[:], start=True, stop=True)

    # input DMAs in consumption order. The sync HWDGE ring carries only
    # the startup-critical slices (first weight block, first v tile in
    # row-halves, second weight block) so the first matmuls are gated by
    # ~400KB, not by the full stream; everything else rides gpsimd SWDGE
    # (outputs join sync later). HBM + SDMA engines are shared across
    # rings, so keeping the early sync traffic minimal matters.
    wt_tile(0, 0)
    v_tile(0, 0, 0)
    load_wt_block(0, 0, 0, 0, nc.sync)          # (ci0, s0): 96KB
    load_v(0, 0, 0, rows=(0, 34), eng=nc.sync)  # nt0-1 rows: 278KB
    load_v(0, 0, 0, rows=(34, VR), eng=nc.sync)
    load_wt_block(0, 0, 1, 0, nc.sync)          # (ci1, s0)
    load_v(0, 1, 0)
    for s in range(1, NS):
        load_wt_block(0, 0, 0, s, nc.sync)
        load_wt_block(0, 0, 1, s, nc.sync)
        load_v(0, 0, s)
        load_v(0, 1, s)
    load_wt(0, 1)
    load_v(1, 0, 0)
    load_v(1, 1, 0)
    load_wt(1, 0)
    for s in range(1, NS):
        load_v(1, 0, s)
        load_v(1, 1, s)
    load_wt(1, 1)

    # ---- winograd-domain GEMM: M[s] = sum_{ci,ky} U[ci,s,ky].T @ V[s] ------
    def drain(b, co, s, nt, ps, last=False):
        # steady state: DVE cast + sync HWDGE out. For the final block the
        # cast/DMA chains split across vector/scalar engines and the two
        # HWDGE rings so the tail is two half-length chains, not one.
        ot = outp.tile([128, RPT * TX], BF16, tag="ot")
        if last and nt % 2 == 1:
            nc.scalar.copy(ot[:], ps[:])
            dma_eng = nc.scalar
        else:
            nc.vector.tensor_copy(ot[:], ps[:])
            dma_eng = nc.sync
        dma_eng.dma_start(
            out=out_dram[b, co, s, :, nt * RPT * TX:(nt + 1) * RPT * TX],
            in_=ot[:])

    def conv(b, co, last=False):
        for s in range(NS):
            ps = [psconv.tile([128, RPT * TX], F32, tag="ps",
                              name=f"ps{b}_{co}_{s}_{nt}")
                  for nt in range(NT)]
            for ci in range(CI):
                for ky in range(KY):
                    lhsT = w_T[b][co][:, ((ci * NS + s) * KY + ky) * 128:
                                      ((ci * NS + s) * KY + ky + 1) * 128]
                    for nt in range(NT):
                        r0 = nt * RPT + ky
                        rhs = v_t[b][ci][s][:, r0 * TX:(r0 + RPT) * TX]
                        nc.tensor.matmul(
                            ps[nt][:], lhsT, rhs,
                            start=(ci == 0 and ky == 0),
                            stop=(ci == CI - 1 and ky == KY - 1))
            for nt in range(NT):
                drain(b, co, s, nt, ps[nt], last=(last and s == NS - 1))

    for b in range(B_LOC):
        for co in range(CO):
            conv(b, co, last=(b == B_LOC - 1 and co == CO - 1))


_NC_CACHE = {}


def _get_nc(repeat=1):
    key = repeat
    if key not in _NC_CACHE:
        _NC_CACHE[key] = _build_nc(repeat)
    return _NC_CACHE[key]


def _prep_host(fmap, mod, kernel_mod, weights):
    """Host-side fp32 weight math + winograd transforms (F(2,3) along x)."""
    B = fmap.shape[0]
    # softmax over the NK base kernels
    e = np.exp(kernel_mod - kernel_mod.max(axis=-1, keepdims=True))
    attn = (e / e.sum(axis=-1, keepdims=True)).astype(np.float32)   # [B, NK]
    w = np.einsum('bn,noikl->boikl', attn, weights)     # [B, O, C, 3, 3]
    w = w * (mod[:, None, :, None, None] + 1.0)
    denom = np.clip((w * w).sum(axis=(2, 3, 4), keepdims=True), 1e-8, None)
    w = w / np.sqrt(denom)
    # weight transform U = G g along kx: [B, O, C, ky, s]
    U = np.stack([w[..., 0],
                  0.5 * (w[..., 0] + w[..., 1] + w[..., 2]),
                  0.5 * (w[..., 0] - w[..., 1] + w[..., 2]),
                  w[..., 2]], axis=-1)
    # lhsT layout: [b, co, i(128p), ci, s, ky, o(128)]
    wt = U.reshape(B, CO, 128, CI, 128, KY, NS)
    wt = wt.transpose(0, 1, 4, 3, 6, 5, 2)       # [b, co, i, ci, s, ky, o]
    wt = np.ascontiguousarray(wt).reshape(B, CO, 128, WCOLS).astype(BF16_NP)
    # input transform V[s] = B^T d along padded cols, rows padded for ky
    dp = np.zeros((B, C, VR, W + 2), dtype=np.float32)
    dp[:, :, 1:H + 1, 1:W + 1] = fmap
    V = np.stack([dp[..., 0:2 * TX:2] - dp[..., 2:2 * TX + 2:2],
                  dp[..., 1:2 * TX + 1:2] + dp[..., 2:2 * TX + 2:2],
                  dp[..., 2:2 * TX + 2:2] - dp[..., 1:2 * TX + 1:2],
                  dp[..., 1:2 * TX + 1:2] - dp[..., 3:2 * TX + 3:2]],
                 axis=2)                          # [B, C, s, VR, TX]
    V = V.reshape(B, CI, 128, NS, VR * TX).transpose(0, 1, 3, 2, 4)
    V = np.ascontiguousarray(V).astype(BF16_NP)   # [B, CI, s, 128, VCOLS]
    return wt, V


def _make_in_maps(wt, V):
    in_maps = []
    for c in range(N_CORES):
        s = slice(c * B_LOC, (c + 1) * B_LOC)
        in_maps.append({
            "wt": np.ascontiguousarray(wt[s]),
            "v": np.ascontiguousarray(V[s]),
        })
    return in_maps


def kernel(fmap, mod, kernel_mod, weights, _trace=False):
    fmap = np.asarray(fmap, dtype=np.float32)
    mod = np.asarray(mod, dtype=np.float32)
    kernel_mod = np.asarray(kernel_mod, dtype=np.float32)
    weights = np.asarray(weights, dtype=np.float32)

    wt, V = _prep_host(fmap, mod, kernel_mod, weights)
    nc = _get_nc()
    in_maps = _make_in_maps(wt, V)
    res = run_bass_kernel_spmd(nc, in_maps, list(range(N_CORES)), trace=_trace)
    B = fmap.shape[0]
    M = np.concatenate([res.results[c]["out"] for c in range(N_CORES)],
                       axis=0).astype(np.float32)
    M = M.reshape(B, CO, NS, 128, H, TX)          # [b, co, s, o, y, tx]
    out = np.empty((B, CO, 128, H, W), dtype=np.float32)
    out[..., 0::2] = M[:, :, 0] + M[:, :, 1] + M[:, :, 2]
    out[..., 1::2] = M[:, :, 1] - M[:, :, 2] - M[:, :, 3]
    out = out.reshape(B, O, H, W)
    if _trace:
        kernel.last_results = res
    return out


# revision 12
# speedup vs baseline: 1.2006x; 1.0658x over previous
"""AdaptiveConv2DMod kernel for 8 TRN2 NeuronCores.

Data-parallel over batch: B=16 -> 2 samples per core.

All transforms run host-side in fp32 numpy (mod/kernel_mod/weights are
host-visible); the device is a pure Winograd-domain batched GEMM:

- Weight math (softmax kernel mix, (1+mod) modulation, demod rsqrt) and
  the F(2,3) 1D Winograd weight transform U = G g (along kx) happen on
  host; each core gets its two samples' U pre-transposed to the matmul
  lhsT layout [b, co, i(128p), ci, s(4), ky(3), o(128)] bf16.
- The fmap is padded (rows+cols) and column-transformed on host into
  V[s] = B^T d (4 Winograd points per 2 output columns), shipped as
  [b, ci, s, ch(128p), 66 rows, 32 tx] bf16.
- Device: 384 matmuls M[s] += U[s,ky].T @ V[s] (shifted rows give the
  direct-ky accumulation; bf16 in / fp32 PSUM), 1.5x less PE work than
  direct 3x3 conv. M drains PSUM -> bf16 SBUF on DVE -> DMA out.
- Host: output transform out = A^T M (3-term combines) in fp32.

PE floor: 384 x 128x128x512 matmuls ~= 82us. HAM warmup dummies keep
the PE clock gate at 8/8 before real work. Weights ride the sync HWDGE
ring, V rides gpsimd SWDGE, M output rides sync behind the weights.
Loop s -> ci -> ky -> nt(4) gives 4-matmul ldweights runs (deduped)
and per-s PSUM drains that overlap the next s-block.
"""

from contextlib import ExitStack

import ml_dtypes
import numpy as np

import concourse.bass as bass
import concourse.mybir as mybir
import concourse.tile as tile
from concourse import bacc
from concourse.bass_utils import run_bass_kernel_spmd

F32 = mybir.dt.float32
BF16 = mybir.dt.bfloat16
BF16_NP = ml_dtypes.bfloat16

N_CORES = 8
B_LOC = 2          # samples per core
C = 256            # input channels (I)
O = 256            # output channels
H = W = 64
NK = 4             # num base kernels
CI = 2             # input channel chunks of 128
CO = 2             # output channel chunks of 128
NS = 4             # winograd points per 2 output cols
KY = 3             # direct taps along y
TX = W // 2        # winograd tiles per row
VR = H + 2         # padded rows in V
NT = 4             # row tiles (16 rows x 32 tx = 512 free)
RPT = H // NT      # rows per tile
WCOLS = CI * NS * KY * 128   # wt free size (3072)
VCOLS = VR * TX              # v free size (2112)


def _build_nc(repeat=1):
    nc = bacc.Bacc("TRN2", target_bir_lowering=False, debug=False,
                   num_devices=N_CORES)
    wt = nc.declare_dram_parameter("wt", [B_LOC, CO, 128, WCOLS],
                                   BF16, isOutput=False)
    v = nc.declare_dram_parameter("v", [B_LOC, CI, NS, 128, VCOLS],
                                  BF16, isOutput=False)
    out = nc.declare_dram_parameter("out", [B_LOC, CO, NS, 128, H * TX],
                                    BF16, isOutput=True)

    with ExitStack() as ctx:
        tc = ctx.enter_context(tile.TileContext(nc))
        pools = _make_pools(ctx, tc)
        for _ in range(repeat):
            _build_body(tc, pools, wt.ap(), v.ap(), out.ap())
    _dedupe_ldweights(nc)
    nc.compile()
    return nc


def _dedupe_ldweights(nc):
    """Remove PE weight reloads that are byte-identical to the previous
    Ldweights and carry no semaphore waits/updates (the split emits one
    Ldweights per matmul even when the stationary operand is unchanged)."""
    removed = 0
    pe = mybir.EngineType.PE
    for blk in nc.main_func.blocks:
        last_key = None
        keep = []
        for inst in blk.instructions:
            tn = type(inst).__name__
            eng = getattr(inst, "engine", None)
            if tn == "InstLdweights":
                key = repr(inst.ins)
                if (key == last_key and inst.sync_info is None):
                    removed += 1
                    continue
                last_key = key
            elif tn == "InstMatmult":
                pass
            elif eng == pe:
                last_key = None
            keep.append(inst)
        blk.instructions[:] = keep
    return removed


def _make_pools(ctx, tc):
    return {
        "wt": ctx.enter_context(tc.tile_pool(name="wt", bufs=B_LOC * CO)),
        "v": ctx.enter_context(tc.tile_pool(name="v", bufs=B_LOC * CI * NS)),
        "outp": ctx.enter_context(tc.tile_pool(name="outp", bufs=8)),
        "psconv": ctx.enter_context(
            tc.tile_pool(name="psconv", bufs=8, space="PSUM")),
    }


def _build_body(tc, pools, wt_dram, v_dram, out_dram):
    nc = tc.nc
    wtp = pools["wt"]
    vp = pools["v"]
    outp = pools["outp"]
    psconv = pools["psconv"]

    w_T = [[None] * CO for _ in range(B_LOC)]
    v_t = [[[None] * NS for _ in range(CI)] for _ in range(B_LOC)]

    CW = NS * KY * 128        # wt cols per ci block
    SW = KY * 128             # wt cols per (ci, s) block

    def wt_tile(b, co):
        t = wtp.tile([128, WCOLS], BF16, tag="wt", name=f"wT{b}_{co}")
        w_T[b][co] = t
        return t

    def load_wt_block(b, co, ci, s, eng):
        c0 = ci * CW + s * SW
        eng.dma_start(out=w_T[b][co][:, c0:c0 + SW],
                      in_=wt_dram[b, co, :, c0:c0 + SW])

    def load_wt(b, co, eng=None):
        t = wt_tile(b, co)
        (eng or nc.gpsimd).dma_start(out=t[:], in_=wt_dram[b, co])

    def v_tile(b, ci, s):
        t = vp.tile([128, VCOLS], BF16, tag="v", name=f"v{b}_{ci}_{s}")
        v_t[b][ci][s] = t
        return t

    def load_v(b, ci, s, rows=None, eng=None):
        t = v_t[b][ci][s] if v_t[b][ci][s] is not None else v_tile(b, ci, s)
        r0, r1 = rows if rows is not None else (0, VR)
        (eng or nc.gpsimd).dma_start(
            out=t[:, r0 * TX:r1 * TX],
            in_=v_dram[b, ci, s, :, r0 * TX:r1 * TX])

    # HAM warmup: dummy matmuls keep PE busy from kernel start so the
    # clock gate is at 8/8 when the first real matmul issues (needs
    # ~3.4us of sustained PE busy; 8 cold matmuls ~= 3.4us). The dummy
    # PSUM slot is released before conv(0,0) claims its 8th bank.
    wz = wtp.tile([128, 512], BF16, tag="wz", bufs=1)
    nc.gpsimd.memset(wz[:], 0.0)
    psd = psconv.tile([128, 512], F32, tag="ps", name="psdummy")
    for _ in range(8):
        nc.tensor.matmul(psd[:], wz[:, 0:128], wz[:], start=True, stop=True)

    # ALL input DMAs ride the sync HWDGE ring in exact consumption order:
    # the ring is FIFO, so issue order IS priority - the startup-critical
    # first blocks are never queued behind speculative stream traffic
    # (SDMA engines round-robin rings at packet granularity with no QoS,
    # so a second ring would steal ~half the bandwidth from the critical
    # path). Outputs ride gpsimd SWDGE; only the final drain block joins
    # sync, long after the input queue drained.
    wt_tile(0, 0)
    v_tile(0, 0, 0)
    load_wt_block(0, 0, 0, 0, nc.sync)          # (ci0, s0): 96KB
    load_v(0, 0, 0, rows=(0, 34), eng=nc.sync)  # nt0-1 rows: 278KB
    load_v(0, 0, 0, rows=(34, VR), eng=nc.sync)
    load_wt_block(0, 0, 1, 0, nc.sync)          # (ci1, s0)
    load_v(0, 1, 0, eng=nc.sync)
    for s in range(1, NS):
        load_wt_block(0, 0, 0, s, nc.sync)
        load_wt_block(0, 0, 1, s, nc.sync)
        load_v(0, 0, s, eng=nc.sync)
        load_v(0, 1, s, eng=nc.sync)
    load_wt(0, 1, eng=nc.sync)
    load_v(1, 0, 0, eng=nc.sync)
    load_v(1, 1, 0, eng=nc.sync)
    load_wt(1, 0, eng=nc.sync)
    for s in range(1, NS):
        load_v(1, 0, s, eng=nc.sync)
        load_v(1, 1, s, eng=nc.sync)
    load_wt(1, 1, eng=nc.sync)

    # ---- winograd-domain GEMM: M[s] = sum_{ci,ky} U[ci,s,ky].T @ V[s] ------
    def drain(b, co, s, nt, ps, last=False):
        # steady state: DVE cast + sync HWDGE out. For the final block the
        # cast/DMA chains split across vector/scalar engines and the two
        # HWDGE rings so the tail is two half-length chains, not one.
        ot = outp.tile([128, RPT * TX], BF16, tag="ot")
        if last and nt % 2 == 1:
            nc.scalar.copy(ot[:], ps[:])
            dma_eng = nc.scalar
        else:
            nc.vector.tensor_copy(ot[:], ps[:])
            dma_eng = nc.sync if last else nc.gpsimd
        dma_eng.dma_start(
            out=out_dram[b, co, s, :, nt * RPT * TX:(nt + 1) * RPT * TX],
            in_=ot[:])

    def conv(b, co, last=False):
        for s in range(NS):
            ps = [psconv.tile([128, RPT * TX], F32, tag="ps",
                              name=f"ps{b}_{co}_{s}_{nt}")
                  for nt in range(NT)]
            for ci in range(CI):
                for ky in range(KY):
                    lhsT = w_T[b][co][:, ((ci * NS + s) * KY + ky) * 128:
                                      ((ci * NS + s) * KY + ky + 1) * 128]
                    for nt in range(NT):
                        r0 = nt * RPT + ky
                        rhs = v_t[b][ci][s][:, r0 * TX:(r0 + RPT) * TX]
                        nc.tensor.matmul(
                            ps[nt][:], lhsT, rhs,
                            start=(ci == 0 and ky == 0),
                            stop=(ci == CI - 1 and ky == KY - 1))
            for nt in range(NT):
                drain(b, co, s, nt, ps[nt], last=(last and s == NS - 1))

    for b in range(B_LOC):
        for co in range(CO):
            conv(b, co, last=(b == B_LOC - 1 and co == CO - 1))


_NC_CACHE = {}


def _get_nc(repeat=1):
    key = repeat
    if key not in _NC_CACHE:
        _NC_CACHE[key] = _build_nc(repeat)
    return _NC_CACHE[key]


def _prep_host(fmap, mod, kernel_mod, weights):
    """Host-side fp32 weight math + winograd transforms (F(2,3) along x)."""
    B = fmap.shape[0]
    # softmax over the NK base kernels
    e = np.exp(kernel_mod - kernel_mod.max(axis=-1, keepdims=True))
    attn = (e / e.sum(axis=-1, keepdims=True)).astype(np.float32)   # [B, NK]
    w = np.einsum('bn,noikl->boikl', attn, weights)     # [B, O, C, 3, 3]
    w = w * (mod[:, None, :, None, None] + 1.0)
    denom = np.clip((w * w).sum(axis=(2, 3, 4), keepdims=True), 1e-8, None)
    w = w / np.sqrt(denom)
    # weight transform U = G g along kx: [B, O, C, ky, s]
    U = np.stack([w[..., 0],
                  0.5 * (w[..., 0] + w[..., 1] + w[..., 2]),
                  0.5 * (w[..., 0] - w[..., 1] + w[..., 2]),
                  w[..., 2]], axis=-1)
    # lhsT layout: [b, co, i(128p), ci, s, ky, o(128)]
    wt = U.reshape(B, CO, 128, CI, 128, KY, NS)
    wt = wt.transpose(0, 1, 4, 3, 6, 5, 2)       # [b, co, i, ci, s, ky, o]
    wt = np.ascontiguousarray(wt).reshape(B, CO, 128, WCOLS).astype(BF16_NP)
    # input transform V[s] = B^T d along padded cols, rows padded for ky
    dp = np.zeros((B, C, VR, W + 2), dtype=np.float32)
    dp[:, :, 1:H + 1, 1:W + 1] = fmap
    V = np.stack([dp[..., 0:2 * TX:2] - dp[..., 2:2 * TX + 2:2],
                  dp[..., 1:2 * TX + 1:2] + dp[..., 2:2 * TX + 2:2],
                  dp[..., 2:2 * TX + 2:2] - dp[..., 1:2 * TX + 1:2],
                  dp[..., 1:2 * TX + 1:2] - dp[..., 3:2 * TX + 3:2]],
                 axis=2)                          # [B, C, s, VR, TX]
    V = V.reshape(B, CI, 128, NS, VR * TX).transpose(0, 1, 3, 2, 4)
    V = np.ascontiguousarray(V).astype(BF16_NP)   # [B, CI, s, 128, VCOLS]
    return wt, V


def _make_in_maps(wt, V):
    in_maps = []
    for c in range(N_CORES):
        s = slice(c * B_LOC, (c + 1) * B_LOC)
        in_maps.append({
            "wt": np.ascontiguousarray(wt[s]),
            "v": np.ascontiguousarray(V[s]),
        })
    return in_maps


def kernel(fmap, mod, kernel_mod, weights, _trace=False):
    fmap = np.asarray(fmap, dtype=np.float32)
    mod = np.asarray(mod, dtype=np.float32)
    kernel_mod = np.asarray(kernel_mod, dtype=np.float32)
    weights = np.asarray(weights, dtype=np.float32)

    wt, V = _prep_host(fmap, mod, kernel_mod, weights)
    nc = _get_nc()
    in_maps = _make_in_maps(wt, V)
    res = run_bass_kernel_spmd(nc, in_maps, list(range(N_CORES)), trace=_trace)
    B = fmap.shape[0]
    M = np.concatenate([res.results[c]["out"] for c in range(N_CORES)],
                       axis=0).astype(np.float32)
    M = M.reshape(B, CO, NS, 128, H, TX)          # [b, co, s, o, y, tx]
    out = np.empty((B, CO, 128, H, W), dtype=np.float32)
    out[..., 0::2] = M[:, :, 0] + M[:, :, 1] + M[:, :, 2]
    out[..., 1::2] = M[:, :, 1] - M[:, :, 2] - M[:, :, 3]
    out = out.reshape(B, O, H, W)
    if _trace:
        kernel.last_results = res
    return out


# revision 14
# speedup vs baseline: 1.4241x; 1.1862x over previous
"""AdaptiveConv2DMod kernel for 8 TRN2 NeuronCores.

Data-parallel over batch: B=16 -> 2 samples per core.

All transforms run host-side in fp32 numpy (mod/kernel_mod/weights are
host-visible); the device is a pure Winograd-domain batched GEMM:

- Weight math (softmax kernel mix, (1+mod) modulation, demod rsqrt) and
  the F(4,3) 1D Winograd weight transform U = G g (along kx) happen on
  host; each core gets its two samples' U pre-transposed to the matmul
  lhsT layout [b, co, i(128p), s(6), ci, ky, o(128)] fp16.
- The fmap is padded (rows+cols) and column-transformed on host into
  V[s] = B^T d (6 Winograd points per 4 output columns), shipped as
  [b, ci, s, ch(128p), 66 rows, 16 tx] fp16.
- Device: 288 matmuls M[s] += U[s,ky].T @ V[s] (shifted rows give the
  direct-ky accumulation; fp16 in / fp32 PSUM) - 2x less PE work than
  direct 3x3 conv. fp16 (not bf16) keeps the Winograd error at ~1.2e-3
  (bf16 would be ~9e-3); the PE runs 16-bit dtypes at the same rate.
- M drains PSUM -> fp16 SBUF on DVE -> DMA out; the host applies the
  output transform out = A^T M (6-term combines) in fp32.

PE floor: 288 x 128x128x512 matmuls ~= 62us. HAM warmup dummies keep
the PE clock gate at 8/8 before real work. ALL input DMAs ride the sync
HWDGE ring in exact consumption order - the ring is FIFO, so issue
order IS priority and startup-critical blocks are never queued behind
stream traffic (SDMA engines round-robin rings at packet granularity
with no QoS). Outputs ride gpsimd SWDGE; only the final drain block
joins sync, long after the input queue drained.
"""

from contextlib import ExitStack

import numpy as np

import concourse.bass as bass
import concourse.mybir as mybir
import concourse.tile as tile
from concourse import bacc
from concourse.bass_utils import run_bass_kernel_spmd

F32 = mybir.dt.float32
FP16 = mybir.dt.float16
FP16_NP = np.float16

N_CORES = 8
B_LOC = 2          # samples per core
C = 256            # input channels (I)
O = 256            # output channels
H = W = 64
NK = 4             # num base kernels
CI = 2             # input channel chunks of 128
CO = 2             # output channel chunks of 128
NS = 6             # winograd points per 4 output cols
KY = 3             # direct taps along y
TX = W // 4        # winograd tiles per row (16)
VR = H + 2         # padded rows in V
NT = 2             # row tiles (32 rows x 16 tx = 512 free)
RPT = H // NT      # rows per tile (32)
WCOLS = NS * CI * KY * 128   # wt free size (4608)
SW = CI * KY * 128           # wt cols per s block (768)
VCOLS = VR * TX              # v free size (1056)
VSPLIT = 34                  # row split for s0 v tiles (nt0 needs rows 0..33)

# F(4,3) transform matrices (points 0, -1, 1, -1/2, 1/2, inf)
BT = np.array([[4, 0, -5, 0, 1, 0],
               [0, -4, -4, 1, 1, 0],
               [0, 4, -4, -1, 1, 0],
               [0, -2, -1, 2, 1, 0],
               [0, 2, -1, -2, 1, 0],
               [0, 4, 0, -5, 0, 1]], dtype=np.float32)
G = np.array([[1 / 4, 0, 0],
              [-1 / 6, -1 / 6, -1 / 6],
              [-1 / 6, 1 / 6, -1 / 6],
              [1 / 24, 1 / 12, 1 / 6],
              [1 / 24, -1 / 12, 1 / 6],
              [0, 0, 1]], dtype=np.float32)
AT = np.array([[1, 1, 1, 1, 1, 0],
               [0, 1, -1, 2, -2, 0],
               [0, 1, 1, 4, 4, 0],
               [0, 1, -1, 8, -8, 1]], dtype=np.float32)


def _build_nc(repeat=1):
    nc = bacc.Bacc("TRN2", target_bir_lowering=False, debug=False,
                   num_devices=N_CORES)
    wt = nc.declare_dram_parameter("wt", [B_LOC, CO, 128, WCOLS],
                                   FP16, isOutput=False)
    v = nc.declare_dram_parameter("v", [B_LOC, CI, NS, 128, VCOLS],
                                  FP16, isOutput=False)
    out = nc.declare_dram_parameter("out", [B_LOC, CO, NS, 128, H * TX],
                                    FP16, isOutput=True)

    with ExitStack() as ctx:
        tc = ctx.enter_context(tile.TileContext(nc))
        pools = _make_pools(ctx, tc)
        for _ in range(repeat):
            _build_body(tc, pools, wt.ap(), v.ap(), out.ap())
    _dedupe_ldweights(nc)
    nc.compile()
    return nc


def _dedupe_ldweights(nc):
    """Remove PE weight reloads that are byte-identical to the previous
    Ldweights and carry no semaphore waits/updates (the split emits one
    Ldweights per matmul even when the stationary operand is unchanged)."""
    removed = 0
    pe = mybir.EngineType.PE
    for blk in nc.main_func.blocks:
        last_key = None
        keep = []
        for inst in blk.instructions:
            tn = type(inst).__name__
            eng = getattr(inst, "engine", None)
            if tn == "InstLdweights":
                key = repr(inst.ins)
                if (key == last_key and inst.sync_info is None):
                    removed += 1
                    continue
                last_key = key
            elif tn == "InstMatmult":
                pass
            elif eng == pe:
                last_key = None
            keep.append(inst)
        blk.instructions[:] = keep
    return removed


def _make_pools(ctx, tc):
    return {
        "wt": ctx.enter_context(tc.tile_pool(name="wt", bufs=B_LOC * CO)),
        "v": ctx.enter_context(
            tc.tile_pool(name="v", bufs=B_LOC * CI * NS)),
        "outp": ctx.enter_context(tc.tile_pool(name="outp", bufs=8)),
        "psconv": ctx.enter_context(
            tc.tile_pool(name="psconv", bufs=8, space="PSUM")),
    }


def _build_body(tc, pools, wt_dram, v_dram, out_dram):
    nc = tc.nc
    wtp = pools["wt"]
    vp = pools["v"]
    outp = pools["outp"]
    psconv = pools["psconv"]

    w_T = [[None] * CO for _ in range(B_LOC)]
    v_t = [[[None] * NS for _ in range(CI)] for _ in range(B_LOC)]

    def wt_tile(b, co):
        t = wtp.tile([128, WCOLS], FP16, tag="wt", name=f"wT{b}_{co}")
        w_T[b][co] = t
        return t

    def load_wt_block(b, co, s):
        nc.sync.dma_start(out=w_T[b][co][:, s * SW:(s + 1) * SW],
                          in_=wt_dram[b, co, :, s * SW:(s + 1) * SW])

    def load_wt(b, co):
        t = wt_tile(b, co)
        nc.sync.dma_start(out=t[:], in_=wt_dram[b, co])

    def v_tile(b, ci, s):
        t = vp.tile([128, VCOLS], FP16, tag="v", name=f"v{b}_{ci}_{s}")
        v_t[b][ci][s] = t
        return t

    def load_v(b, ci, s, rows=None):
        t = v_t[b][ci][s] if v_t[b][ci][s] is not None else v_tile(b, ci, s)
        r0, r1 = rows if rows is not None else (0, VR)
        nc.sync.dma_start(
            out=t[:, r0 * TX:r1 * TX],
            in_=v_dram[b, ci, s, :, r0 * TX:r1 * TX])

    # HAM warmup: dummy matmuls keep PE busy from kernel start so the
    # clock gate is at 8/8 when the first real matmul issues (needs
    # ~3.4us of sustained PE busy; 8 cold matmuls ~= 3.4us). The dummy
    # PSUM slot is released before the conv claims its 8th bank.
    wz = wtp.tile([128, 512], FP16, tag="wz", bufs=1)
    nc.gpsimd.memset(wz[:], 0.0)
    psd = psconv.tile([128, 512], F32, tag="ps", name="psdummy")
    for _ in range(8):
        nc.tensor.matmul(psd[:], wz[:, 0:128], wz[:], start=True, stop=True)

    # input DMAs on sync HWDGE in exact consumption order
    wt_tile(0, 0)
    v_tile(0, 0, 0)
    v_tile(0, 1, 0)
    load_wt_block(0, 0, 0)                 # s0 weights: 192KB
    load_v(0, 0, 0, rows=(0, VSPLIT))      # s0 ci0 nt0 rows: 139KB
    load_v(0, 1, 0, rows=(0, VSPLIT))
    load_v(0, 0, 0, rows=(VSPLIT, VR))
    load_v(0, 1, 0, rows=(VSPLIT, VR))
    for s in range(1, NS):
        load_wt_block(0, 0, s)
        load_v(0, 0, s)
        load_v(0, 1, s)
    load_wt(0, 1)
    for s in range(NS):
        load_v(1, 0, s)
        load_v(1, 1, s)
        if s == 0:
            load_wt(1, 0)
        if s == 1:
            load_wt(1, 1)

    # ---- winograd-domain GEMM: M[s] = sum_{ci,ky} U[s,ky].T @ V[s] ------
    def drain(b, co, s, nt, ps, last=False):
        # steady state: DVE cast + gpsimd SWDGE out. For the final block
        # the cast/DMA chains split across vector/scalar engines and the
        # two HWDGE rings so the tail is two half-length chains.
        ot = outp.tile([128, RPT * TX], FP16, tag="ot")
        if last and nt % 2 == 1:
            nc.scalar.copy(ot[:], ps[:])
            dma_eng = nc.scalar
        else:
            nc.vector.tensor_copy(ot[:], ps[:])
            dma_eng = nc.sync if last else nc.gpsimd
        dma_eng.dma_start(
            out=out_dram[b, co, s, :, nt * RPT * TX:(nt + 1) * RPT * TX],
            in_=ot[:])

    def conv(b, co, last=False):
        for s in range(NS):
            ps = [psconv.tile([128, RPT * TX], F32, tag="ps",
                              name=f"ps{b}_{co}_{s}_{nt}")
                  for nt in range(NT)]
            for ci in range(CI):
                for ky in range(KY):
                    c0 = ((s * CI + ci) * KY + ky) * 128
                    lhsT = w_T[b][co][:, c0:c0 + 128]
                    for nt in range(NT):
                        r0 = nt * RPT + ky
                        rhs = v_t[b][ci][s][:, r0 * TX:(r0 + RPT) * TX]
                        nc.tensor.matmul(
                            ps[nt][:], lhsT, rhs,
                            start=(ci == 0 and ky == 0),
                            stop=(ci == CI - 1 and ky == KY - 1))
            for nt in range(NT):
                drain(b, co, s, nt, ps[nt], last=(last and s == NS - 1))

    for b in range(B_LOC):
        for co in range(CO):
            conv(b, co, last=(b == B_LOC - 1 and co == CO - 1))


_NC_CACHE = {}


def _get_nc(repeat=1):
    key = repeat
    if key not in _NC_CACHE:
        _NC_CACHE[key] = _build_nc(repeat)
    return _NC_CACHE[key]


def _prep_host(fmap, mod, kernel_mod, weights):
    """Host-side fp32 weight math + winograd transforms (F(4,3) along x)."""
    B = fmap.shape[0]
    # softmax over the NK base kernels
    e = np.exp(kernel_mod - kernel_mod.max(axis=-1, keepdims=True))
    attn = (e / e.sum(axis=-1, keepdims=True)).astype(np.float32)   # [B, NK]
    w = np.einsum('bn,noikl->boikl', attn, weights)     # [B, O, C, 3, 3]
    w = w * (mod[:, None, :, None, None] + 1.0)
    denom = np.clip((w * w).sum(axis=(2, 3, 4), keepdims=True), 1e-8, None)
    w = w / np.sqrt(denom)
    # weight transform U[s, ky] = sum_kx G[s, kx] w[..., ky, kx]
    U = np.einsum('sx,boikx->boiks', G, w)              # [B, O, C, ky, s]
    # lhsT layout: [b, co, i(128p), s, ci, ky, o(128)]
    wt = U.reshape(B, CO, 128, CI, 128, KY, NS)
    wt = wt.transpose(0, 1, 4, 6, 3, 5, 2)       # [b, co, i, s, ci, ky, o]
    wt = np.ascontiguousarray(wt).reshape(B, CO, 128, WCOLS).astype(FP16_NP)
    # input transform V[s] = B^T d along padded cols, rows padded for ky
    dp = np.zeros((B, C, VR, W + 2), dtype=np.float32)
    dp[:, :, 1:H + 1, 1:W + 1] = fmap
    cols = np.arange(TX) * 4
    V = np.zeros((B, C, NS, VR, TX), dtype=np.float32)
    for s in range(NS):
        for vv in range(NS):
            cf = BT[s, vv]
            if cf:
                V[:, :, s] += cf * dp[:, :, :, cols + vv]
    V = V.reshape(B, CI, 128, NS, VR * TX).transpose(0, 1, 3, 2, 4)
    V = np.ascontiguousarray(V).astype(FP16_NP)   # [B, CI, s, 128, VCOLS]
    return wt, V


def _make_in_maps(wt, V):
    in_maps = []
    for c in range(N_CORES):
        s = slice(c * B_LOC, (c + 1) * B_LOC)
        in_maps.append({
            "wt": np.ascontiguousarray(wt[s]),
            "v": np.ascontiguousarray(V[s]),
        })
    return in_maps


def kernel(fmap, mod, kernel_mod, weights, _trace=False):
    fmap = np.asarray(fmap, dtype=np.float32)
    mod = np.asarray(mod, dtype=np.float32)
    kernel_mod = np.asarray(kernel_mod, dtype=np.float32)
    weights = np.asarray(weights, dtype=np.float32)

    wt, V = _prep_host(fmap, mod, kernel_mod, weights)
    nc = _get_nc()
    in_maps = _make_in_maps(wt, V)
    res = run_bass_kernel_spmd(nc, in_maps, list(range(N_CORES)), trace=_trace)
    B = fmap.shape[0]
    M = np.concatenate([res.results[c]["out"] for c in range(N_CORES)],
                       axis=0).astype(np.float32)
    M = M.reshape(B, CO, NS, 128, H, TX)          # [b, co, s, o, y, tx]
    out = np.empty((B, CO, 128, H, W), dtype=np.float32)
    for q in range(4):
        acc = AT[q, 0] * M[:, :, 0]
        for s in range(1, NS):
            if AT[q, s]:
                acc = acc + AT[q, s] * M[:, :, s]
        out[..., q::4] = acc
    out = out.reshape(B, O, H, W)
    if _trace:
        kernel.last_results = res
    return out
